# revision 1
# baseline (speedup 1.0000x reference)
"""JointNetwork Trainium2 kernel.

out[b,t,u,f] = (audio[b] @ W[:H])[t,f] + (label[b] @ W[H:])[u,f] + bias[f]

Sharding: data-parallel over B — B=8 batch elements map 1:1 onto the 8
NeuronCores; no communication.

Per-core plan (memory regime: the 64 MiB fp32 output write at the ~358 GB/s
per-core HBM cap dominates; measured ~205 us vs ~187 us write roofline):
  1. Inputs stream in as bf16 (halves load bytes).  PE transposes build the
     [H, T]/[H, U] stationary operands; bf16 matmuls compute a = audio@Wa
     [256,1024] and l = label@Wl + b [64,1024] (PSUM accumulates fp32).
  2. Streams 128 output tiles of [128 rows, 1024]: rows = 2 t-values x 64
     u-values.  PE broadcasts a-rows into PSUM with one-hot selection
     matmuls (bf16: 1 cyc/row + fast weight load); l_tiled [l; l] is
     materialized once in fp32.
  3. Two copy paths drain PSUM, balancing engines: most tiles take DVE
     tensor_add(psum, l_tiled) -> SBUF; every third tile keeps the l-add on
     PE (selL matmuls) with an ACT copy.  Out-DMAs split across both HWDGE
     rings (sync + scalar) and sustain ~350 GB/s to HBM.
"""

import numpy as np

B, T, U, H, F = 8, 256, 64, 512, 1024
N_CORES = 8
NTILES = (T * U) // 128  # 128 output tiles of [128, F] per core
TPC = T // 128  # t-chunks (a row chunks)
KC = H // 128  # contraction chunks for projections

# broadcast-stage matmul dtype: "f32r" (fast, fp32 bits single-pass),
# "f32" (exact, 4x slower), "bf16" (fast, rounds a/l to bf16)
BCAST = "bf16"
OUT_BUFS = 20
PSUM_BUFS = 4
ACT_EVERY = 3


def _build_nc():
    import concourse.bacc as bacc
    import concourse.mybir as mybir
    import concourse.tile as tile

    f32 = mybir.dt.float32
    f32r = mybir.dt.float32r
    bf16 = mybir.dt.bfloat16
    bdt = {"f32r": f32r, "f32": f32, "bf16": bf16}[BCAST]

    nc = bacc.Bacc("TRN2", target_bir_lowering=False, debug=False)

    audio_d = nc.dram_tensor("audio", [T, H], bf16, kind="ExternalInput")
    label_d = nc.dram_tensor("label", [U, H], bf16, kind="ExternalInput")
    w_d = nc.dram_tensor("w", [2 * H, F], bf16, kind="ExternalInput")
    bias_d = nc.dram_tensor("bias", [1, F], bf16, kind="ExternalInput")
    sela_d = nc.dram_tensor("sela", [128, 64 * 128], bdt, kind="ExternalInput")
    sell_d = nc.dram_tensor("sell", [U, 128], bdt, kind="ExternalInput")
    ident_d = nc.dram_tensor("ident", [128, 128], bf16, kind="ExternalInput")
    ones_d = nc.dram_tensor("ones", [1, U], bf16, kind="ExternalInput")
    out_d = nc.dram_tensor("out", [T * U, F], f32, kind="ExternalOutput")

    out_view = out_d.rearrange("(n p) f -> n p f", p=128)

    with tile.TileContext(nc) as tc:
        with (
            tc.tile_pool(name="const", bufs=1) as cpool,
            tc.tile_pool(name="w", bufs=1) as wpool,
            tc.tile_pool(name="proj", bufs=1) as ppool,
            tc.tile_pool(name="psum", bufs=PSUM_BUFS, space="PSUM") as ps_pool,
            tc.tile_pool(name="out", bufs=OUT_BUFS) as opool,
        ):
            # ---- load inputs ----
            ident = cpool.tile([128, 128], bf16)
            nc.scalar.dma_start(out=ident[:], in_=ident_d[:])
            ones = cpool.tile([1, U], bf16)
            nc.scalar.dma_start(out=ones[:], in_=ones_d[:])
            bias = cpool.tile([1, F], bf16)
            nc.scalar.dma_start(out=bias[:], in_=bias_d[:])
            sela = cpool.tile([128, 64 * 128], bdt)
            nc.gpsimd.dma_start(out=sela[:], in_=sela_d[:])
            sell = cpool.tile([U, 128], bdt)
            nc.scalar.dma_start(out=sell[:], in_=sell_d[:])

            # wl half (needed first, feeds l/lt) on the scalar ring; wa on sync
            wtiles = [None] * (2 * KC)
            for k in range(KC, 2 * KC):
                wt = wpool.tile([128, F], bf16, tag=f"w{k}", name=f"w{k}")
                nc.scalar.dma_start(out=wt[:], in_=w_d[k * 128 : (k + 1) * 128, :])
                wtiles[k] = wt
            for k in range(KC):
                wt = wpool.tile([128, F], bf16, tag=f"w{k}", name=f"w{k}")
                nc.sync.dma_start(out=wt[:], in_=w_d[k * 128 : (k + 1) * 128, :])
                wtiles[k] = wt

            audio_sb = []
            for c in range(TPC):
                at = ppool.tile([128, H], bf16, tag=f"audio{c}", name=f"audio{c}")
                nc.scalar.dma_start(out=at[:], in_=audio_d[c * 128 : (c + 1) * 128, :])
                audio_sb.append(at)
            label_sb = ppool.tile([U, H], bf16, tag="label")
            nc.scalar.dma_start(out=label_sb[:], in_=label_d[:])

            # ---- transposes: audioT[k] = audio[:, k*128:+128].T  [128, T] ----
            audio_t = [ppool.tile([128, T], bf16, tag=f"at{k}", name=f"at{k}") for k in range(KC)]
            label_t = [ppool.tile([128, U], bf16, tag=f"lt{k}", name=f"lt{k}") for k in range(KC)]
            for k in range(KC):
                pt = ps_pool.tile([128, 2 * F], bf16, tag="ps", name="pt")
                nc.tensor.transpose(
                    pt[:, 0:U], label_sb[:, k * 128 : (k + 1) * 128], ident[0:U, 0:U]
                )
                nc.scalar.copy(out=label_t[k][:], in_=pt[:, 0:U])
                for c in range(TPC):
                    pt = ps_pool.tile([128, 2 * F], bf16, tag="ps", name="pt")
                    nc.tensor.transpose(
                        pt[:, 0:128], audio_sb[c][:, k * 128 : (k + 1) * 128], ident[:]
                    )
                    nc.scalar.copy(
                        out=audio_t[k][:, c * 128 : (c + 1) * 128], in_=pt[:, 0:128]
                    )

            # ---- projections (fp32) ----
            l_sb = ppool.tile([U, F], bdt, tag="l")
            pl = ps_pool.tile([128, F], f32, tag="ps", name="pl")
            for nh in range(2):
                sl = slice(nh * 512, (nh + 1) * 512)
                for k in range(KC):
                    nc.tensor.matmul(
                        pl[0:U, sl],
                        lhsT=label_t[k][:, 0:U],
                        rhs=wtiles[KC + k][:, sl],
                        start=(k == 0),
                        stop=False,
                    )
                nc.tensor.matmul(
                    pl[0:U, sl],
                    lhsT=ones[:, 0:U],
                    rhs=bias[:, sl],
                    start=False,
                    stop=True,
                )
            nc.scalar.copy(out=l_sb[:], in_=pl[0:U, :])

            # l_tiled [128, F] = [l; l] (fp32, includes bias) for DVE adds
            lt_sb = ppool.tile([128, F], f32, tag="lt")
            plt = ps_pool.tile([128, F], f32, tag="ps", name="plt")
            for nh in range(2):
                sl = slice(nh * 512, (nh + 1) * 512)
                nc.tensor.matmul(
                    plt[:, sl], lhsT=sell[:, :], rhs=l_sb[:, sl], start=True, stop=True
                )
            nc.scalar.copy(out=lt_sb[:], in_=plt[:])

            a_sb = [ppool.tile([128, F], bdt, tag=f"a{c}", name=f"a{c}") for c in range(TPC)]
            for c in range(TPC):
                pa = ps_pool.tile([128, F], f32, tag="ps", name="pa")
                for nh in range(2):
                    sl = slice(nh * 512, (nh + 1) * 512)
                    for k in range(KC):
                        nc.tensor.matmul(
                            pa[:, sl],
                            lhsT=audio_t[k][:, c * 128 : (c + 1) * 128],
                            rhs=wtiles[k][:, sl],
                            start=(k == 0),
                            stop=(k == KC - 1),
                        )
                nc.scalar.copy(out=a_sb[c][:], in_=pa[:])


            # ---- broadcast-add stream ----
            for i in range(NTILES):
                c, j = divmod(i, 64)
                act_tile = i % ACT_EVERY == 0
                po = ps_pool.tile([128, F], f32, tag="ps", name="po")
                for nh in range(2):
                    sl = slice(nh * 512, (nh + 1) * 512)
                    nc.tensor.matmul(
                        po[:, sl],
                        lhsT=sela[:, j * 128 : (j + 1) * 128],
                        rhs=a_sb[c][:, sl],
                        start=True,
                        stop=not act_tile,
                    )
                ot = opool.tile([128, F], f32)
                if act_tile:
                    # PE adds l_tiled via selL matmuls; ACT copies out
                    for nh in range(2):
                        sl = slice(nh * 512, (nh + 1) * 512)
                        nc.tensor.matmul(
                            po[:, sl],
                            lhsT=sell[:, :],
                            rhs=l_sb[:, sl],
                            start=False,
                            stop=True,
                        )
                    nc.scalar.copy(out=ot[:], in_=po[:])
                    nc.scalar.dma_start(out=out_view[i], in_=ot[:])
                else:
                    # DVE adds l_tiled during the PSUM->SBUF move
                    nc.vector.tensor_add(out=ot[:], in0=po[:], in1=lt_sb[:])
                    nc.sync.dma_start(out=out_view[i], in_=ot[:])

    nc.compile()
    return nc


_NC = None


def _get_nc():
    global _NC
    if _NC is None:
        _NC = _build_nc()
    return _NC


def _host_consts():
    import ml_dtypes

    seldt = {"bf16": ml_dtypes.bfloat16, "f32r": np.float32, "f32": np.float32}[BCAST]
    sela = np.zeros((128, 64 * 128), dtype=seldt)
    for j in range(64):
        for m in range(128):
            sela[2 * j + (1 if m >= 64 else 0), j * 128 + m] = 1.0
    sell = np.zeros((U, 128), dtype=seldt)
    for m in range(128):
        sell[m % U, m] = 1.0
    ident = np.eye(128, dtype=np.float32)
    ones = np.ones((1, U), dtype=np.float32)
    return sela, sell, ident, ones


def _in_maps(audio_vector, label_vector, W, b):
    import ml_dtypes

    bf = ml_dtypes.bfloat16
    sela, sell, ident, ones = _host_consts()
    wb = np.ascontiguousarray(W).astype(bf)
    maps = []
    for i in range(N_CORES):
        maps.append(
            {
                "audio": np.ascontiguousarray(audio_vector[i]).astype(bf),
                "label": np.ascontiguousarray(label_vector[i]).astype(bf),
                "w": wb,
                "bias": np.ascontiguousarray(b).astype(bf).reshape(1, F),
                "sela": sela,
                "sell": sell,
                "ident": ident.astype(bf),
                "ones": ones.astype(bf),
            }
        )
    return maps


def _run(in_maps, **kw):
    from concourse.bass_utils import run_bass_kernel_spmd

    nc = _get_nc()
    return run_bass_kernel_spmd(nc, in_maps, core_ids=list(range(N_CORES)), **kw)


def kernel(audio_vector, label_vector, W, b):
    res = _run(_in_maps(audio_vector, label_vector, W, b))
    out = np.stack([res.results[i]["out"].reshape(T, U, F) for i in range(N_CORES)])
    return out



# revision 8
# speedup vs baseline: 1.7673x; 1.7673x over previous
"""JointNetwork Trainium2 kernel.

out[b,t,u,f] = (audio[b] @ W[:H])[t,f] + (label[b] @ W[H:])[u,f] + b[f]

Sharding: data-parallel over B — B=8 batch elements map 1:1 onto the 8
NeuronCores; no communication.

Memory regime: the output write dominates.  Output is stored bf16 (rel-err
budget 2e-2; bf16 adds ~2e-3) in u-major layout [U*T, F] so each SBUF tile
[128 t-rows, F] for a fixed u lands contiguously; host restores [T,U,F] via
a transposed view and upcasts to fp32.  32 MiB/core at ~330 GB/s ≈ 100 us.

Per-core pipeline:
  1. Host pre-transposes audio/label to [H, T]/[H, U] bf16 (no on-device
     transposes).  PE computes a = audio@Wa -> a_sb [2][128, F] bf16 and
     l = label@Wl + bias -> l_sb [U, F] bf16.
  2. For each u: PE broadcasts l_sb[u] to 128 partitions (K=1 matmul with
     ones lhsT, 2x 512-col, ~432 ns); ACT drains PSUM -> lbu bf16.
  3. For c in {0,1}: DVE tensor_add(a_sb[c], lbu) -> bf16 out tile in
     2x_1P mode (~594 ns); both c-tiles share one [128, 2F] SBUF buffer
     DMA'd as a single 512 KiB transfer, alternating sync/scalar rings.
"""

import numpy as np

B, T, U, H, F = 8, 256, 64, 512, 1024
N_CORES = 8
KC = H // 128  # contraction chunks
TPC = T // 128  # t-chunks

OUT_BUFS = 10  # [128, 2F] bf16 = 512 KiB each
LBU_BUFS = 4
PSUM_BUFS = 3


def _build_nc():
    import concourse.bacc as bacc
    import concourse.mybir as mybir
    import concourse.tile as tile

    f32 = mybir.dt.float32
    bf16 = mybir.dt.bfloat16

    nc = bacc.Bacc("TRN2", target_bir_lowering=False, debug=False)

    audio_t_d = nc.dram_tensor("audio_t", [H, T], bf16, kind="ExternalInput")
    label_t_d = nc.dram_tensor("label_t", [H, U], bf16, kind="ExternalInput")
    w_d = nc.dram_tensor("w", [2 * H, F], bf16, kind="ExternalInput")
    bias_d = nc.dram_tensor("bias", [1, F], bf16, kind="ExternalInput")
    ones_d = nc.dram_tensor("ones", [1, 128], bf16, kind="ExternalInput")
    sel_d = nc.dram_tensor("sel", [U, U * 128], bf16, kind="ExternalInput")
    out_d = nc.dram_tensor("out", [U * T, F], bf16, kind="ExternalOutput")

    # [u] -> [128 partitions, 2 t-chunks, F]: partition p, (b, f) maps to
    # DRAM row u*T + b*128 + p, col f
    out_view = out_d.rearrange("(u b p) f -> u p b f", b=TPC, p=128)

    with tile.TileContext(nc) as tc:
        with (
            tc.tile_pool(name="const", bufs=1) as cpool,
            tc.tile_pool(name="w", bufs=1) as wpool,
            tc.tile_pool(name="proj", bufs=1) as ppool,
            tc.tile_pool(name="psum", bufs=PSUM_BUFS, space="PSUM") as ps_pool,
            tc.tile_pool(name="lbu", bufs=LBU_BUFS) as lpool,
            tc.tile_pool(name="out", bufs=OUT_BUFS) as opool,
        ):
            # ---- load inputs: label/Wl path on scalar ring (feeds l first),
            # audio/Wa on sync ring ----
            ones = cpool.tile([1, 128], bf16)
            nc.scalar.dma_start(out=ones[:], in_=ones_d[:])
            sel = cpool.tile([U, U * 128], bf16)
            nc.gpsimd.dma_start(out=sel[:], in_=sel_d[:])
            bias = cpool.tile([1, F], bf16)
            nc.scalar.dma_start(out=bias[:], in_=bias_d[:])
            lt = []
            for k in range(KC):
                t_ = ppool.tile([128, U], bf16, tag=f"lt{k}", name=f"lt{k}")
                nc.scalar.dma_start(out=t_[:], in_=label_t_d[k * 128 : (k + 1) * 128, :])
                lt.append(t_)
            wtiles = [None] * (2 * KC)
            for k in range(KC, 2 * KC):
                wt = wpool.tile([128, F], bf16, tag=f"w{k}", name=f"w{k}")
                nc.scalar.dma_start(out=wt[:], in_=w_d[k * 128 : (k + 1) * 128, :])
                wtiles[k] = wt
            at = []
            for k in range(KC):
                t_ = ppool.tile([128, T], bf16, tag=f"at{k}", name=f"at{k}")
                nc.sync.dma_start(out=t_[:], in_=audio_t_d[k * 128 : (k + 1) * 128, :])
                at.append(t_)
            for k in range(KC):
                wt = wpool.tile([128, F], bf16, tag=f"w{k}", name=f"w{k}")
                nc.sync.dma_start(out=wt[:], in_=w_d[k * 128 : (k + 1) * 128, :])
                wtiles[k] = wt

            # ---- l projection: l = label @ Wl + bias  [U, F] ----
            l_sb = ppool.tile([U, F], bf16, tag="l")
            pl = ps_pool.tile([128, F], f32, tag="ps", name="pl")
            for nh in range(2):
                sl = slice(nh * 512, (nh + 1) * 512)
                for k in range(KC):
                    nc.tensor.matmul(
                        pl[0:U, sl],
                        lhsT=lt[k][:, 0:U],
                        rhs=wtiles[KC + k][:, sl],
                        start=(k == 0),
                        stop=False,
                    )
                nc.tensor.matmul(
                    pl[0:U, sl],
                    lhsT=ones[:, 0:U],
                    rhs=bias[:, sl],
                    start=False,
                    stop=True,
                )
            nc.scalar.copy(out=l_sb[:], in_=pl[0:U, :])

            # ---- a projection: a = audio @ Wa  [2][128, F] ----
            a_sb = []
            for c in range(TPC):
                pa = ps_pool.tile([128, F], f32, tag="ps", name=f"pa{c}")
                for nh in range(2):
                    sl = slice(nh * 512, (nh + 1) * 512)
                    for k in range(KC):
                        nc.tensor.matmul(
                            pa[:, sl],
                            lhsT=at[k][:, c * 128 : (c + 1) * 128],
                            rhs=wtiles[k][:, sl],
                            start=(k == 0),
                            stop=(k == KC - 1),
                        )
                t_ = ppool.tile([128, F], bf16, tag=f"a{c}", name=f"a{c}")
                nc.scalar.copy(out=t_[:], in_=pa[:])
                a_sb.append(t_)

            # ---- broadcast-add stream ----
            for u in range(U):
                plu = ps_pool.tile([128, F], f32, tag="ps", name=f"plu{u}")
                for nh in range(2):
                    sl = slice(nh * 512, (nh + 1) * 512)
                    nc.tensor.matmul(
                        plu[:, sl],
                        lhsT=sel[:, u * 128 : (u + 1) * 128],
                        rhs=l_sb[:, sl],
                        start=True,
                        stop=True,
                    )
                lbu = lpool.tile([128, F], bf16)
                nc.scalar.copy(out=lbu[:], in_=plu[:])

                ot = opool.tile([128, TPC * F], bf16)
                for c in range(TPC):
                    nc.vector.tensor_add(
                        out=ot[:, c * F : (c + 1) * F], in0=a_sb[c][:], in1=lbu[:]
                    )
                eng = nc.sync if u % 2 == 0 else nc.scalar
                eng.dma_start(out=out_view[u], in_=ot[:])

    nc.compile()
    return nc


_NC = None


def _get_nc():
    global _NC
    if _NC is None:
        _NC = _build_nc()
    return _NC


def _in_maps(audio_vector, label_vector, W, b):
    import ml_dtypes

    bf = ml_dtypes.bfloat16
    wb = np.ascontiguousarray(W).astype(bf)
    bias = np.ascontiguousarray(b).astype(bf).reshape(1, F)
    ones = np.ones((1, 128), dtype=bf)
    sel = np.zeros((U, U * 128), dtype=bf)
    for u in range(U):
        sel[u, u * 128 : (u + 1) * 128] = 1.0
    maps = []
    for i in range(N_CORES):
        maps.append(
            {
                "audio_t": np.ascontiguousarray(audio_vector[i].T).astype(bf),
                "label_t": np.ascontiguousarray(label_vector[i].T).astype(bf),
                "w": wb,
                "bias": bias,
                "ones": ones,
                "sel": sel,
            }
        )
    return maps


def _run(in_maps, **kw):
    from concourse.bass_utils import run_bass_kernel_spmd

    nc = _get_nc()
    return run_bass_kernel_spmd(nc, in_maps, core_ids=list(range(N_CORES)), **kw)


def kernel(audio_vector, label_vector, W, b):
    res = _run(_in_maps(audio_vector, label_vector, W, b))
    out = np.stack(
        [
            np.asarray(res.results[i]["out"])
            .reshape(U, T, F)
            .transpose(1, 0, 2)
            for i in range(N_CORES)
        ]
    )
    return out.astype(np.float32)
